# revision 28
# baseline (speedup 1.0000x reference)
"""CenterLoss kernel for 8 Trainium2 NeuronCores.

loss = mean(distmat * onehot(labels)) over a (B, C) distmat where
distmat[i, j] = ||x_i - c_j||^2.  The mask selects exactly one element
per row, so  loss = (1/(B*C)) * sum_i ||x_i - c_{labels[i]}||^2.

Strategy: data-parallel over batch.  Each of the 8 cores takes 512 rows
of x, gathers its 512 center rows from the (replicated) centers table
with one ucode dma_gather, computes sum((x-g)^2) on the vector engine,
and writes a [128,1] per-partition partial sum.  The host sums the
8x128 partials in float64 and divides by B*C.

Raw Bass (no Tile): the toolchain allows at most one semaphore wait
per compute instruction, so cross-engine deps are taken with
standalone wait_ge instructions instead of instruction-attached waits.
"""

import sys

if "/opt/trn_rl_repo" not in sys.path:
    sys.path.insert(0, "/opt/trn_rl_repo")

import numpy as np

import concourse.bass as bass
from concourse import mybir
from concourse.bacc import Bacc

NCORES = 8
B = 4096
D = 128
C = 20000
P = 128
BS = B // NCORES          # 512 rows per core
N = BS // P               # 4 rows per partition

USE_DMA_GATHER = True


def build_bass() -> bass.Bass:
    nc = Bacc() if USE_DMA_GATHER else bass.Bass()
    x = nc.declare_dram_parameter("x", [BS, D], mybir.dt.float32, isOutput=False)
    idx_dt = mybir.dt.int16 if USE_DMA_GATHER else mybir.dt.int32
    idx_dram_shape = [P, BS // 16] if USE_DMA_GATHER else [BS]
    idx = nc.declare_dram_parameter("idx", idx_dram_shape, idx_dt, isOutput=False)
    centers = nc.declare_dram_parameter(
        "centers", [C, D], mybir.dt.float32, isOutput=False
    )
    out = nc.declare_dram_parameter("out", [P, 1], mybir.dt.float32, isOutput=True)

    idx_shape = [P, BS // 16] if USE_DMA_GATHER else [P, N]

    with (
        nc.sbuf_tensor(idx_shape, idx_dt) as idx_t,
        nc.sbuf_tensor([P, N, D], mybir.dt.float32) as x_t,
        nc.sbuf_tensor([P, N, D], mybir.dt.float32) as g_t,
        nc.sbuf_tensor([P, N * D], mybir.dt.float32) as d_t,
        nc.sbuf_tensor([P, N * D], mybir.dt.float32) as sq_t,
        nc.sbuf_tensor([P, 1], mybir.dt.float32) as red_t,
        nc.semaphore("idx_sem") as idx_sem,
        nc.semaphore("x_sem") as x_sem,
        nc.semaphore("g_sem") as g_sem,
        nc.semaphore("v_sem") as v_sem,
        nc.semaphore("done_sem") as done_sem,
        nc.Block() as block,
    ):

        @block.sync
        def _(sync):
            if USE_DMA_GATHER:
                # idx already host-wrapped: idx[k%16, k//16] = labels[k]
                sync.dma_start(out=idx_t[:], in_=idx[:]).then_inc(idx_sem, 16)
                # row i=n*128+p -> partition p, slot n (dma_gather layout)
                sync.dma_start(
                    out=x_t[:], in_=x[:].rearrange("(n p) d -> p n d", p=P)
                ).then_inc(x_sem, 16)
            else:
                sync.dma_start(
                    out=idx_t[:], in_=idx[:].rearrange("(p n) -> p n", p=P)
                ).then_inc(idx_sem, 16)
                sync.dma_start(
                    out=x_t[:], in_=x[:].rearrange("(p n) d -> p n d", p=P)
                ).then_inc(x_sem, 16)
            sync.wait_ge(v_sem, 3)
            # No wait on done_sem: the Block-end queue drain guarantees the
            # store lands before kernel completion.
            sync.dma_start(out=out[:], in_=red_t[:]).then_inc(done_sem, 16)

        @block.gpsimd
        def _(gpsimd):
            gpsimd.wait_ge(idx_sem, 16)
            if USE_DMA_GATHER:
                gpsimd.dma_gather(
                    out_ap=g_t[:],
                    in_ap=centers[:],
                    idxs_ap=idx_t[:],
                    num_idxs=BS,
                    num_idxs_reg=BS,
                    elem_size=D,
                ).then_inc(g_sem, 16)
            else:
                for n in range(N):
                    gpsimd.indirect_dma_start(
                        out=g_t[:, n, :],
                        out_offset=None,
                        in_=centers[:],
                        in_offset=bass.IndirectOffsetOnAxis(
                            ap=idx_t[:, n : n + 1], axis=0
                        ),
                    ).then_inc(g_sem, 16)

        @block.vector
        def _(vector):
            vector.wait_ge(x_sem, 16)
            vector.wait_ge(g_sem, 16 if USE_DMA_GATHER else 16 * N)
            # The chain sems between DVE ops are free on HW (they overlap
            # the per-op pipeline DRAIN) and keep the race detector happy.
            vector.tensor_tensor(
                out=d_t[:],
                in0=x_t[:].rearrange("p n d -> p (n d)"),
                in1=g_t[:].rearrange("p n d -> p (n d)"),
                op=mybir.AluOpType.subtract,
            ).then_inc(v_sem, 1)
            vector.wait_ge(v_sem, 1)
            vector.tensor_tensor(
                out=sq_t[:], in0=d_t[:], in1=d_t[:], op=mybir.AluOpType.mult
            ).then_inc(v_sem, 1)
            vector.wait_ge(v_sem, 2)
            vector.tensor_reduce(
                out=red_t[:],
                in_=sq_t[:],
                axis=mybir.AxisListType.X,
                op=mybir.AluOpType.add,
            ).then_inc(v_sem, 1)

    if not nc.is_finalized():
        nc.finalize()
    return nc


_NC = None


def _get_nc() -> bass.Bass:
    global _NC
    if _NC is None:
        _NC = build_bass()
    return _NC


def make_in_maps(x, labels, centers):
    x = np.ascontiguousarray(np.asarray(x, dtype=np.float32))
    idx_dt = np.int16 if USE_DMA_GATHER else np.int32
    labels = np.asarray(labels).astype(idx_dt)
    centers = np.ascontiguousarray(np.asarray(centers, dtype=np.float32))
    in_maps = []
    for c in range(NCORES):
        sl = slice(c * BS, (c + 1) * BS)
        idx_c = labels[sl]
        if USE_DMA_GATHER:
            # wrap: idx[k%16, k//16] = labels[k]; partitions 16..127 are
            # zero padding (the gather ucode reads only the first 16).
            wrapped = np.zeros((P, BS // 16), dtype=np.int16)
            wrapped[:16] = idx_c.reshape(BS // 16, 16).T
            idx_c = wrapped
        in_maps.append(
            {
                "x": np.ascontiguousarray(x[sl]),
                "idx": np.ascontiguousarray(idx_c),
                "centers": centers,
            }
        )
    return in_maps


def reduce_outputs(results) -> np.ndarray:
    total = 0.0
    for r in results:
        total += float(np.sum(r["out"].astype(np.float64)))
    return np.array(np.float32(total / (B * C)))


def kernel(x, labels, centers) -> np.ndarray:
    from concourse.bass_utils import run_bass_kernel_spmd

    nc = _get_nc()
    in_maps = make_in_maps(x, labels, centers)
    res = run_bass_kernel_spmd(nc, in_maps, list(range(NCORES)))
    return reduce_outputs(res.results)


# revision 29
# speedup vs baseline: 1.2436x; 1.2436x over previous
"""CenterLoss kernel for 8 Trainium2 NeuronCores.

loss = mean(distmat * onehot(labels)) over a (B, C) distmat where
distmat[i, j] = ||x_i - c_j||^2.  The mask selects exactly one element
per row, so  loss = (1/(B*C)) * sum_i ||x_i - c_{labels[i]}||^2.

Strategy: data-parallel over batch.  Each of the 8 cores takes 512 rows
of x, gathers its 512 center rows from the (replicated) centers table
with one ucode dma_gather, computes sum((x-g)^2) on the vector engine,
and writes a [128,1] per-partition partial sum.  The host sums the
8x128 partials in float64 and divides by B*C.

Raw Bass (no Tile): the toolchain allows at most one semaphore wait
per compute instruction, so cross-engine deps are taken with
standalone wait_ge instructions instead of instruction-attached waits.
"""

import sys

if "/opt/trn_rl_repo" not in sys.path:
    sys.path.insert(0, "/opt/trn_rl_repo")

import numpy as np

import concourse.bass as bass
from concourse import mybir
from concourse.bacc import Bacc

NCORES = 8
B = 4096
D = 128
C = 20000
P = 128
BS = B // NCORES          # 512 rows per core
N = BS // P               # 4 rows per partition

USE_DMA_GATHER = True


def build_bass() -> bass.Bass:
    nc = Bacc() if USE_DMA_GATHER else bass.Bass()
    x = nc.declare_dram_parameter("x", [BS, D], mybir.dt.float32, isOutput=False)
    idx_dt = mybir.dt.int16 if USE_DMA_GATHER else mybir.dt.int32
    idx_dram_shape = [P, BS // 16] if USE_DMA_GATHER else [BS]
    idx = nc.declare_dram_parameter("idx", idx_dram_shape, idx_dt, isOutput=False)
    centers = nc.declare_dram_parameter(
        "centers", [C, D], mybir.dt.float32, isOutput=False
    )
    out = nc.declare_dram_parameter("out", [P, 1], mybir.dt.float32, isOutput=True)

    idx_shape = [P, BS // 16] if USE_DMA_GATHER else [P, N]

    with (
        nc.sbuf_tensor(idx_shape, idx_dt) as idx_t,
        nc.sbuf_tensor([P, N, D], mybir.dt.float32) as x_t,
        nc.sbuf_tensor([P, N, D], mybir.dt.float32) as g_t,
        nc.sbuf_tensor([P, N * D], mybir.dt.float32) as d_t,
        nc.sbuf_tensor([P, N * D], mybir.dt.float32) as sq_t,
        nc.sbuf_tensor([P, 1], mybir.dt.float32) as red_t,
        nc.semaphore("idx_sem") as idx_sem,
        nc.semaphore("x_sem") as x_sem,
        nc.semaphore("g_sem") as g_sem,
        nc.semaphore("v_sem") as v_sem,
        nc.semaphore("done_sem") as done_sem,
        nc.Block() as block,
    ):

        @block.sync
        def _(sync):
            if USE_DMA_GATHER:
                # idx already host-wrapped: idx[k%16, k//16] = labels[k]
                sync.dma_start(out=idx_t[:], in_=idx[:]).then_inc(idx_sem, 16)
                # row i=n*128+p -> partition p, slot n (dma_gather layout)
                sync.dma_start(
                    out=x_t[:], in_=x[:].rearrange("(n p) d -> p n d", p=P)
                ).then_inc(x_sem, 16)
            else:
                sync.dma_start(
                    out=idx_t[:], in_=idx[:].rearrange("(p n) -> p n", p=P)
                ).then_inc(idx_sem, 16)
                sync.dma_start(
                    out=x_t[:], in_=x[:].rearrange("(p n) d -> p n d", p=P)
                ).then_inc(x_sem, 16)
            sync.wait_ge(v_sem, 3)
            # No wait on done_sem: the Block-end queue drain guarantees the
            # store lands before kernel completion.
            sync.dma_start(out=out[:], in_=red_t[:]).then_inc(done_sem, 16)

        @block.gpsimd
        def _(gpsimd):
            gpsimd.wait_ge(idx_sem, 16)
            if USE_DMA_GATHER:
                gpsimd.dma_gather(
                    out_ap=g_t[:],
                    in_ap=centers[:],
                    idxs_ap=idx_t[:],
                    num_idxs=BS,
                    num_idxs_reg=BS,
                    elem_size=D,
                ).then_inc(g_sem, 16)
            else:
                for n in range(N):
                    gpsimd.indirect_dma_start(
                        out=g_t[:, n, :],
                        out_offset=None,
                        in_=centers[:],
                        in_offset=bass.IndirectOffsetOnAxis(
                            ap=idx_t[:, n : n + 1], axis=0
                        ),
                    ).then_inc(g_sem, 16)

        @block.vector
        def _(vector):
            vector.wait_ge(x_sem, 16)
            vector.wait_ge(g_sem, 16 if USE_DMA_GATHER else 16 * N)
            # The chain sems between DVE ops are free on HW (they overlap
            # the per-op pipeline DRAIN) and keep the race detector happy.
            vector.tensor_tensor(
                out=d_t[:],
                in0=x_t[:].rearrange("p n d -> p (n d)"),
                in1=g_t[:].rearrange("p n d -> p (n d)"),
                op=mybir.AluOpType.subtract,
            ).then_inc(v_sem, 1)
            vector.wait_ge(v_sem, 1)
            vector.tensor_tensor(
                out=sq_t[:], in0=d_t[:], in1=d_t[:], op=mybir.AluOpType.mult
            ).then_inc(v_sem, 1)
            vector.wait_ge(v_sem, 2)
            vector.tensor_reduce(
                out=red_t[:],
                in_=sq_t[:],
                axis=mybir.AxisListType.X,
                op=mybir.AluOpType.add,
            ).then_inc(v_sem, 1)

    if not nc.is_finalized():
        nc.finalize()
    return nc


_NC = None


def _get_nc() -> bass.Bass:
    global _NC
    if _NC is None:
        _NC = build_bass()
    return _NC


def make_in_maps(x, labels, centers):
    x = np.ascontiguousarray(np.asarray(x, dtype=np.float32))
    idx_dt = np.int16 if USE_DMA_GATHER else np.int32
    labels = np.asarray(labels).astype(idx_dt)
    centers = np.ascontiguousarray(np.asarray(centers, dtype=np.float32))
    in_maps = []
    for c in range(NCORES):
        sl = slice(c * BS, (c + 1) * BS)
        idx_c = labels[sl]
        if USE_DMA_GATHER:
            # wrap: idx[k%16, k//16] = labels[k], replicated across all
            # 128 partitions (the gather ucode's lanes each read their
            # own partition group).
            idx_c = np.tile(idx_c.reshape(BS // 16, 16).T, (P // 16, 1))
        in_maps.append(
            {
                "x": np.ascontiguousarray(x[sl]),
                "idx": np.ascontiguousarray(idx_c),
                "centers": centers,
            }
        )
    return in_maps


def reduce_outputs(results) -> np.ndarray:
    total = 0.0
    for r in results:
        total += float(np.sum(r["out"].astype(np.float64)))
    return np.array(np.float32(total / (B * C)))


def kernel(x, labels, centers) -> np.ndarray:
    from concourse.bass_utils import run_bass_kernel_spmd

    nc = _get_nc()
    in_maps = make_in_maps(x, labels, centers)
    res = run_bass_kernel_spmd(nc, in_maps, list(range(NCORES)))
    return reduce_outputs(res.results)


# revision 30
# speedup vs baseline: 1.8813x; 1.5128x over previous
"""CenterLoss kernel for 8 Trainium2 NeuronCores.

loss = mean(distmat * onehot(labels)) over a (B, C) distmat where
distmat[i, j] = ||x_i - c_j||^2.  The mask selects exactly one element
per row, so  loss = (1/(B*C)) * sum_i ||x_i - c_{labels[i]}||^2.

Strategy: data-parallel over batch.  Each of the 8 cores takes 512 rows
of x, gathers its 512 center rows from the (replicated) centers table
with one ucode dma_gather, computes sum((x-g)^2) on the vector engine,
and writes a [128,1] per-partition partial sum.  The host sums the
8x128 partials in float64 and divides by B*C.

Raw Bass (no Tile): the toolchain allows at most one semaphore wait
per compute instruction, so cross-engine deps are taken with
standalone wait_ge instructions instead of instruction-attached waits.
"""

import sys

if "/opt/trn_rl_repo" not in sys.path:
    sys.path.insert(0, "/opt/trn_rl_repo")

import numpy as np

import concourse.bass as bass
from concourse import mybir
from concourse.bacc import Bacc

NCORES = 8
B = 4096
D = 128
C = 20000
P = 128
BS = B // NCORES          # 512 rows per core
N = BS // P               # 4 rows per partition

USE_DMA_GATHER = False


def build_bass() -> bass.Bass:
    nc = Bacc() if USE_DMA_GATHER else bass.Bass()
    x = nc.declare_dram_parameter("x", [BS, D], mybir.dt.float32, isOutput=False)
    idx_dt = mybir.dt.int16 if USE_DMA_GATHER else mybir.dt.int32
    idx_dram_shape = [P, BS // 16] if USE_DMA_GATHER else [BS]
    idx = nc.declare_dram_parameter("idx", idx_dram_shape, idx_dt, isOutput=False)
    centers = nc.declare_dram_parameter(
        "centers", [C, D], mybir.dt.float32, isOutput=False
    )
    out = nc.declare_dram_parameter("out", [P, 1], mybir.dt.float32, isOutput=True)

    idx_shape = [P, BS // 16] if USE_DMA_GATHER else [P, N]

    with (
        nc.sbuf_tensor(idx_shape, idx_dt) as idx_t,
        nc.sbuf_tensor([P, N, D], mybir.dt.float32) as x_t,
        nc.sbuf_tensor([P, N, D], mybir.dt.float32) as g_t,
        nc.sbuf_tensor([P, N * D], mybir.dt.float32) as d_t,
        nc.sbuf_tensor([P, N * D], mybir.dt.float32) as sq_t,
        nc.sbuf_tensor([P, 1], mybir.dt.float32) as red_t,
        nc.semaphore("idx_sem") as idx_sem,
        nc.semaphore("x_sem") as x_sem,
        nc.semaphore("g_sem") as g_sem,
        nc.semaphore("v_sem") as v_sem,
        nc.semaphore("done_sem") as done_sem,
        nc.Block() as block,
    ):

        @block.sync
        def _(sync):
            if USE_DMA_GATHER:
                # idx already host-wrapped: idx[k%16, k//16] = labels[k]
                sync.dma_start(out=idx_t[:], in_=idx[:]).then_inc(idx_sem, 16)
                # row i=n*128+p -> partition p, slot n (dma_gather layout)
                sync.dma_start(
                    out=x_t[:], in_=x[:].rearrange("(n p) d -> p n d", p=P)
                ).then_inc(x_sem, 16)
            else:
                sync.dma_start(
                    out=idx_t[:], in_=idx[:].rearrange("(p n) -> p n", p=P)
                ).then_inc(idx_sem, 16)
                sync.dma_start(
                    out=x_t[:], in_=x[:].rearrange("(p n) d -> p n d", p=P)
                ).then_inc(x_sem, 16)
            sync.wait_ge(v_sem, 3)
            # No wait on done_sem: the Block-end queue drain guarantees the
            # store lands before kernel completion.
            sync.dma_start(out=out[:], in_=red_t[:]).then_inc(done_sem, 16)

        @block.gpsimd
        def _(gpsimd):
            gpsimd.wait_ge(idx_sem, 16)
            if USE_DMA_GATHER:
                gpsimd.dma_gather(
                    out_ap=g_t[:],
                    in_ap=centers[:],
                    idxs_ap=idx_t[:],
                    num_idxs=BS,
                    num_idxs_reg=BS,
                    elem_size=D,
                ).then_inc(g_sem, 16)
            else:
                for n in range(N):
                    gpsimd.indirect_dma_start(
                        out=g_t[:, n, :],
                        out_offset=None,
                        in_=centers[:],
                        in_offset=bass.IndirectOffsetOnAxis(
                            ap=idx_t[:, n : n + 1], axis=0
                        ),
                    ).then_inc(g_sem, 16)

        @block.vector
        def _(vector):
            vector.wait_ge(x_sem, 16)
            vector.wait_ge(g_sem, 16 if USE_DMA_GATHER else 16 * N)
            # The chain sems between DVE ops are free on HW (they overlap
            # the per-op pipeline DRAIN) and keep the race detector happy.
            vector.tensor_tensor(
                out=d_t[:],
                in0=x_t[:].rearrange("p n d -> p (n d)"),
                in1=g_t[:].rearrange("p n d -> p (n d)"),
                op=mybir.AluOpType.subtract,
            ).then_inc(v_sem, 1)
            vector.wait_ge(v_sem, 1)
            vector.tensor_tensor(
                out=sq_t[:], in0=d_t[:], in1=d_t[:], op=mybir.AluOpType.mult
            ).then_inc(v_sem, 1)
            vector.wait_ge(v_sem, 2)
            vector.tensor_reduce(
                out=red_t[:],
                in_=sq_t[:],
                axis=mybir.AxisListType.X,
                op=mybir.AluOpType.add,
            ).then_inc(v_sem, 1)

    if not nc.is_finalized():
        nc.finalize()
    return nc


_NC = None


def _get_nc() -> bass.Bass:
    global _NC
    if _NC is None:
        _NC = build_bass()
    return _NC


def make_in_maps(x, labels, centers):
    x = np.ascontiguousarray(np.asarray(x, dtype=np.float32))
    idx_dt = np.int16 if USE_DMA_GATHER else np.int32
    labels = np.asarray(labels).astype(idx_dt)
    centers = np.ascontiguousarray(np.asarray(centers, dtype=np.float32))
    in_maps = []
    for c in range(NCORES):
        sl = slice(c * BS, (c + 1) * BS)
        idx_c = labels[sl]
        if USE_DMA_GATHER:
            # wrap: idx[k%16, k//16] = labels[k], replicated across all
            # 128 partitions (the gather ucode's lanes each read their
            # own partition group).
            idx_c = np.tile(idx_c.reshape(BS // 16, 16).T, (P // 16, 1))
        in_maps.append(
            {
                "x": np.ascontiguousarray(x[sl]),
                "idx": np.ascontiguousarray(idx_c),
                "centers": centers,
            }
        )
    return in_maps


def reduce_outputs(results) -> np.ndarray:
    total = 0.0
    for r in results:
        total += float(np.sum(r["out"].astype(np.float64)))
    return np.array(np.float32(total / (B * C)))


def kernel(x, labels, centers) -> np.ndarray:
    from concourse.bass_utils import run_bass_kernel_spmd

    nc = _get_nc()
    in_maps = make_in_maps(x, labels, centers)
    res = run_bass_kernel_spmd(nc, in_maps, list(range(NCORES)))
    return reduce_outputs(res.results)


# revision 34
# speedup vs baseline: 1.9302x; 1.0260x over previous
"""CenterLoss kernel for 8 Trainium2 NeuronCores.

loss = mean(distmat * onehot(labels)) over a (B, C) distmat where
distmat[i, j] = ||x_i - c_j||^2.  The mask selects exactly one element
per row, so  loss = (1/(B*C)) * sum_i ||x_i - c_{labels[i]}||^2.

Strategy: data-parallel over batch.  Each of the 8 cores takes 512 rows
of x, gathers its 512 center rows from the (replicated) centers table
with indirect DMAs (two halves, pipelined against the vector engine),
computes sum((x-g)^2) per half, and writes a [128,2] partial-sum tile.
The host sums the 8x128x2 partials in float64 and divides by B*C.

Raw Bass (no Tile): the toolchain allows at most one semaphore wait
per compute instruction, so cross-engine deps are taken with
standalone wait_ge instructions instead of instruction-attached waits.
"""

import sys

if "/opt/trn_rl_repo" not in sys.path:
    sys.path.insert(0, "/opt/trn_rl_repo")

import numpy as np

import concourse.bass as bass
from concourse import mybir

NCORES = 8
B = 4096
D = 128
C = 20000
P = 128
BS = B // NCORES          # 512 rows per core
N = BS // P               # 4 rows per partition
H = N // 2                # gathers per half

WARM_SWDGE = True


def build_bass() -> bass.Bass:
    nc = bass.Bass()
    x = nc.declare_dram_parameter("x", [BS, D], mybir.dt.float32, isOutput=False)
    idx = nc.declare_dram_parameter("idx", [BS], mybir.dt.int32, isOutput=False)
    centers = nc.declare_dram_parameter(
        "centers", [C, D], mybir.dt.float32, isOutput=False
    )
    out = nc.declare_dram_parameter("out", [P, 2], mybir.dt.float32, isOutput=True)

    with (
        nc.sbuf_tensor([P, N], mybir.dt.int32) as idx_t,
        nc.sbuf_tensor([P, N, D], mybir.dt.float32) as x_t,
        nc.sbuf_tensor([P, N, D], mybir.dt.float32) as g_t,
        nc.sbuf_tensor([P, N, D], mybir.dt.float32) as d_t,
        nc.sbuf_tensor([P, N, D], mybir.dt.float32) as sq_t,
        nc.sbuf_tensor([P, 2], mybir.dt.float32) as red_t,
        nc.sbuf_tensor([P, 1], mybir.dt.float32) as warm_t,
        nc.semaphore("idx_sem") as idx_sem,
        nc.semaphore("x_sem") as x_sem,
        nc.semaphore("g0_sem") as g0_sem,
        nc.semaphore("g1_sem") as g1_sem,
        nc.semaphore("v_sem") as v_sem,
        nc.semaphore("warm_sem") as warm_sem,
        nc.semaphore("done_sem") as done_sem,
        nc.Block(no_gpsimd_drain=True) as block,
    ):

        @block.sync
        def _(sync):
            sync.dma_start(
                out=idx_t[:], in_=idx[:].rearrange("(p n) -> p n", p=P)
            ).then_inc(idx_sem, 16)
            sync.dma_start(
                out=x_t[:], in_=x[:].rearrange("(p n) d -> p n d", p=P)
            ).then_inc(x_sem, 16)
            sync.wait_ge(v_sem, 6)
            # No wait on done_sem: the Sync queue drain at block end
            # guarantees the store lands before kernel completion.
            sync.dma_start(out=out[:], in_=red_t[:]).then_inc(done_sem, 16)

        @block.gpsimd
        def _(gpsimd):
            if WARM_SWDGE:
                # Tiny DMA to warm the SWDGE path before idx arrives.
                gpsimd.dma_start(out=warm_t[:1, :1], in_=centers[0:1, 0:1]).then_inc(
                    warm_sem, 16
                )
            gpsimd.wait_ge(idx_sem, 16)
            # HW honors only one offset per partition per indirect DMA, so
            # issue N gathers with [P, 1] offset tiles.
            for n in range(N):
                gpsimd.indirect_dma_start(
                    out=g_t[:, n, :],
                    out_offset=None,
                    in_=centers[:],
                    in_offset=bass.IndirectOffsetOnAxis(
                        ap=idx_t[:, n : n + 1], axis=0
                    ),
                ).then_inc(g0_sem if n < H else g1_sem, 16)

        @block.vector
        def _(vector):
            if WARM_SWDGE:
                vector.wait_ge(warm_sem, 16)
            vector.wait_ge(x_sem, 16)
            # Two halves: compute on half h while the gathers for half
            # h+1 are still running.  The chain sems between DVE ops are
            # cheap (they overlap the per-op pipeline DRAIN) and keep the
            # race detector happy.
            for h in range(2):
                lo, hi = h * H, (h + 1) * H
                vector.wait_ge(g0_sem if h == 0 else g1_sem, 16 * H)
                vector.tensor_tensor(
                    out=d_t[:, lo:hi, :].rearrange("p n d -> p (n d)"),
                    in0=x_t[:, lo:hi, :].rearrange("p n d -> p (n d)"),
                    in1=g_t[:, lo:hi, :].rearrange("p n d -> p (n d)"),
                    op=mybir.AluOpType.subtract,
                ).then_inc(v_sem, 1)
                vector.wait_ge(v_sem, 3 * h + 1)
                vector.tensor_tensor(
                    out=sq_t[:, lo:hi, :].rearrange("p n d -> p (n d)"),
                    in0=d_t[:, lo:hi, :].rearrange("p n d -> p (n d)"),
                    in1=d_t[:, lo:hi, :].rearrange("p n d -> p (n d)"),
                    op=mybir.AluOpType.mult,
                ).then_inc(v_sem, 1)
                vector.wait_ge(v_sem, 3 * h + 2)
                vector.tensor_reduce(
                    out=red_t[:, h : h + 1],
                    in_=sq_t[:, lo:hi, :],
                    axis=mybir.AxisListType.XY,
                    op=mybir.AluOpType.add,
                ).then_inc(v_sem, 1)

    if not nc.is_finalized():
        nc.finalize()
    return nc


_NC = None


def _get_nc() -> bass.Bass:
    global _NC
    if _NC is None:
        _NC = build_bass()
    return _NC


def make_in_maps(x, labels, centers):
    x = np.ascontiguousarray(np.asarray(x, dtype=np.float32))
    labels = np.asarray(labels).astype(np.int32)
    centers = np.ascontiguousarray(np.asarray(centers, dtype=np.float32))
    in_maps = []
    for c in range(NCORES):
        sl = slice(c * BS, (c + 1) * BS)
        in_maps.append(
            {
                "x": np.ascontiguousarray(x[sl]),
                "idx": np.ascontiguousarray(labels[sl]),
                "centers": centers,
            }
        )
    return in_maps


def reduce_outputs(results) -> np.ndarray:
    total = 0.0
    for r in results:
        total += float(np.sum(r["out"].astype(np.float64)))
    return np.array(np.float32(total / (B * C)))


def kernel(x, labels, centers) -> np.ndarray:
    from concourse.bass_utils import run_bass_kernel_spmd

    nc = _get_nc()
    in_maps = make_in_maps(x, labels, centers)
    res = run_bass_kernel_spmd(nc, in_maps, list(range(NCORES)))
    return reduce_outputs(res.results)


# revision 35
# speedup vs baseline: 1.9692x; 1.0202x over previous
"""CenterLoss kernel for 8 Trainium2 NeuronCores.

loss = mean(distmat * onehot(labels)) over a (B, C) distmat where
distmat[i, j] = ||x_i - c_j||^2.  The mask selects exactly one element
per row, so  loss = (1/(B*C)) * sum_i ||x_i - c_{labels[i]}||^2.

Strategy: data-parallel over batch.  Each of the 8 cores takes 512 rows
of x, gathers its 512 center rows from the (replicated) centers table
with 4 indirect DMAs (one per 128-row chunk, pipelined against the
vector engine), computes sum((x-g)^2) per chunk via subtract +
fused square-reduce (scalar_tensor_tensor accum), and writes a [128,4]
partial-sum tile.  The host sums the partials in float64 and divides
by B*C.

Raw Bass (no Tile): the toolchain allows at most one semaphore wait
per compute instruction, so cross-engine deps are taken with
standalone wait_ge instructions instead of instruction-attached waits.
"""

import sys

if "/opt/trn_rl_repo" not in sys.path:
    sys.path.insert(0, "/opt/trn_rl_repo")

import numpy as np

import concourse.bass as bass
from concourse import mybir

NCORES = 8
B = 4096
D = 128
C = 20000
P = 128
BS = B // NCORES          # 512 rows per core
N = BS // P               # 4 rows per partition


def build_bass() -> bass.Bass:
    nc = bass.Bass()
    x = nc.declare_dram_parameter("x", [BS, D], mybir.dt.float32, isOutput=False)
    idx = nc.declare_dram_parameter("idx", [BS], mybir.dt.int32, isOutput=False)
    centers = nc.declare_dram_parameter(
        "centers", [C, D], mybir.dt.float32, isOutput=False
    )
    out = nc.declare_dram_parameter("out", [P, N], mybir.dt.float32, isOutput=True)

    with (
        nc.sbuf_tensor([P, N], mybir.dt.int32) as idx_t,
        nc.sbuf_tensor([P, N, D], mybir.dt.float32) as x_t,
        nc.sbuf_tensor([P, N, D], mybir.dt.float32) as g_t,
        nc.sbuf_tensor([P, N, D], mybir.dt.float32) as d_t,
        nc.sbuf_tensor([P, N, D], mybir.dt.float32) as sq_t,
        nc.sbuf_tensor([P, N], mybir.dt.float32) as red_t,
        nc.semaphore("idx_sem") as idx_sem,
        nc.semaphore("x_sem") as x_sem,
        nc.semaphore("ga_sem") as ga_sem,
        nc.semaphore("gb_sem") as gb_sem,
        nc.semaphore("gc_sem") as gc_sem,
        nc.semaphore("gd_sem") as gd_sem,
        nc.semaphore("v_sem") as v_sem,
        nc.semaphore("done_sem") as done_sem,
        nc.Block(no_gpsimd_drain=True) as block,
    ):
        g_sems = [ga_sem, gb_sem, gc_sem, gd_sem]

        @block.sync
        def _(sync):
            sync.dma_start(
                out=idx_t[:], in_=idx[:].rearrange("(p n) -> p n", p=P)
            ).then_inc(idx_sem, 16)
            sync.dma_start(
                out=x_t[:], in_=x[:].rearrange("(p n) d -> p n d", p=P)
            ).then_inc(x_sem, 16)
            sync.wait_ge(v_sem, 2 * N)
            # No wait on done_sem: the Sync queue drain at block end
            # guarantees the store lands before kernel completion.
            sync.dma_start(out=out[:], in_=red_t[:]).then_inc(done_sem, 16)

        @block.gpsimd
        def _(gpsimd):
            gpsimd.wait_ge(idx_sem, 16)
            # HW honors only one offset per partition per indirect DMA, so
            # issue N gathers with [P, 1] offset tiles.
            for n in range(N):
                gpsimd.indirect_dma_start(
                    out=g_t[:, n, :],
                    out_offset=None,
                    in_=centers[:],
                    in_offset=bass.IndirectOffsetOnAxis(
                        ap=idx_t[:, n : n + 1], axis=0
                    ),
                ).then_inc(g_sems[n], 16)

        @block.vector
        def _(vector):
            vector.wait_ge(x_sem, 16)
            # Chunk n computes while chunk n+1's gather is in flight.
            # The v_sem chain between dependent DVE ops is cheap (it
            # overlaps the per-op pipeline DRAIN) and keeps the race
            # detector happy.
            for n in range(N):
                vector.wait_ge(g_sems[n], 16)
                vector.tensor_tensor(
                    out=d_t[:, n, :],
                    in0=x_t[:, n, :],
                    in1=g_t[:, n, :],
                    op=mybir.AluOpType.subtract,
                ).then_inc(v_sem, 1)
                vector.wait_ge(v_sem, 2 * n + 1)
                # sq = (d + 0) * d ; accum = sum(sq)  — fused square+reduce
                vector.scalar_tensor_tensor(
                    out=sq_t[:, n, :],
                    in0=d_t[:, n, :],
                    scalar=0.0,
                    in1=d_t[:, n, :],
                    op0=mybir.AluOpType.add,
                    op1=mybir.AluOpType.mult,
                    accum_out=red_t[:, n : n + 1],
                ).then_inc(v_sem, 1)

    if not nc.is_finalized():
        nc.finalize()
    return nc


_NC = None


def _get_nc() -> bass.Bass:
    global _NC
    if _NC is None:
        _NC = build_bass()
    return _NC


def make_in_maps(x, labels, centers):
    x = np.ascontiguousarray(np.asarray(x, dtype=np.float32))
    labels = np.asarray(labels).astype(np.int32)
    centers = np.ascontiguousarray(np.asarray(centers, dtype=np.float32))
    in_maps = []
    for c in range(NCORES):
        sl = slice(c * BS, (c + 1) * BS)
        in_maps.append(
            {
                "x": np.ascontiguousarray(x[sl]),
                "idx": np.ascontiguousarray(labels[sl]),
                "centers": centers,
            }
        )
    return in_maps


def reduce_outputs(results) -> np.ndarray:
    total = 0.0
    for r in results:
        total += float(np.sum(r["out"].astype(np.float64)))
    return np.array(np.float32(total / (B * C)))


def kernel(x, labels, centers) -> np.ndarray:
    from concourse.bass_utils import run_bass_kernel_spmd

    nc = _get_nc()
    in_maps = make_in_maps(x, labels, centers)
    res = run_bass_kernel_spmd(nc, in_maps, list(range(NCORES)))
    return reduce_outputs(res.results)
